# revision 5
# baseline (speedup 1.0000x reference)
"""LoRA Linear kernel for Trainium2, 8 NeuronCores — v17.

out = x @ (W + lora_A @ lora_B)^T + bias for x [4,2048,4096],
W [4096,4096], lora_A [4096,16], lora_B [16,4096].

Sharding: pure column-parallel (8-way out_features); every core sees all
8192 tokens and 512 out features.  O_LOC=512 = one PSUM bank per token
tile -> 8 independent accumulation chains, ~2 us tail.

Numerics (numpy-emulated, matches HW to 4 digits): Wtot folded on host;
NF8=8 of 32 k-slices as e4m3 DoubleRow pairs (FS=8 symmetric pow-2
scales), 24 slices in fp16, fp16 output.  rel_l2 = 1.892e-2 < 2e-2.

v17 scheduling (from v16 trace analysis):
- Per-queue DMA rates are asymmetric (measured: scalar ~72 GB/s,
  sync ~88, gpsimd ~165; distinct start offsets).  Lead items are
  assigned earliest-projected-finish across 4 queues (scalar, sync,
  gpsimd, vector) with criticality-ordered priorities; the PE lead
  stream is generated by a build-time greedy that paces 8 chains
  against modeled arrivals.
- Modeled arrival gaps are bridged with zero-matmuls (0 x 0 accumulated
  into a mid-chain PSUM bank adds 0.0): keeps HAM at K=8/8 through the
  lead phase instead of re-throttling to 1.2 GHz.
- Steady phase: xb rides the fast gpsimd queue alone, x8 on sync,
  out on scalar; last two stores are split in half and pushed on two
  queues to shorten the tail.
"""

import ml_dtypes
import numpy as np

import concourse.bacc as bacc
import concourse.mybir as mybir
import concourse.tile as tile
from concourse.bass_utils import run_bass_kernel_spmd

IN_F = 4096
OUT_F = 4096
BATCH, SEQ = 4, 2048
M_TOT = BATCH * SEQ          # 8192 tokens
OG = 8                       # pure out-feature sharding
O_LOC = OUT_F // OG          # 512 out features per core = one PSUM bank
P = 128
KI = IN_F // P               # 32 contraction slices
NF8 = 8                      # k-slices done as e4m3 DoubleRow (even)
KB = KI - NF8                # 24 k-slices at fp16
NP = NF8 // 2                # DoubleRow pairs
MT = M_TOT // P              # 64 token tiles per core
NLEAD = 8                    # lead chains = all 8 PSUM banks
NCHUNK = 6                   # lead xb split into 6 chunks of 4 slices
CK = KB // NCHUNK
FS = 8.0                     # fp8 symmetric scale: x/FS, W*FS
NWARM = 30                   # HAM warm-up matmuls (N=128, dep-free)

F32 = mybir.dt.float32
F16 = mybir.dt.float16
E4 = mybir.dt.float8e4
DR = mybir.MatmulPerfMode.DoubleRow

_cache = {}


def _lead_schedule():
    """Assign lead DMA items to 4 queues (earliest projected finish,
    measured per-queue rates) and greedily pace the 8 lead chains
    against modeled arrivals.  Emits filler markers for modeled gaps.

    Returns (queue_lists, pe_ops):
      queue_lists: per-queue ordered item lists
      pe_ops: ('dr',t,j) | ('bf',t,ki) | ('fill',n)
    """
    # priority-ordered items
    items = []
    items += [('x8', 0), ('w8', 0), ('x8', 1), ('w8', 1), ('w8', 2), ('w8', 3)]
    items += [('x8', t) for t in range(2, NLEAD)]
    wb_next = 0

    def wb_run(n):
        nonlocal wb_next
        out = [('wb', k) for k in range(wb_next, min(wb_next + n, KB))]
        wb_next += len(out)
        return out

    xbc = lambda t: [('xb', t, c) for c in range(NCHUNK)]
    items += xbc(0) + wb_run(2)
    items += xbc(1) + wb_run(2)
    items += xbc(2) + wb_run(3)
    items += xbc(3) + wb_run(3)
    items += xbc(4) + wb_run(3)
    items += xbc(5) + wb_run(4)
    items += xbc(6) + wb_run(4)
    items += xbc(7) + wb_run(KB) + [('bias',)]

    KIB = {'x8': 128, 'w8': 128, 'xb': 256 * 3 // NCHUNK, 'wb': 128, 'bias': 256}
    # measured per-queue (start us, us-per-KiB): scalar ~72 GB/s,
    # sync ~88, gpsimd (software DGE) ~165 but starts late.
    QCFG = [(7.6, 0.01389), (7.3, 0.01136), (10.0, 0.00606)]
    NQ = len(QCFG)
    qt = [s for s, _ in QCFG]
    qlists = [[] for _ in QCFG]
    arr = {}
    for it in items:
        kib = KIB[it[0]]
        fins = [max(qt[q], QCFG[q][0]) + kib * QCFG[q][1] for q in range(NQ)]
        q = min(range(NQ), key=lambda i: fins[i])
        qt[q] = fins[q]
        qlists[q].append(it)
        arr[it] = fins[q]

    # --- PE greedy (first-runnable chain in order; fillers on gaps) ---
    T_DR, T_BF, T_FILL = 0.241, 0.213, 0.107
    t_pe = 7.4 + NWARM * T_FILL
    ptr = [0] * NLEAD
    NOPS = NP + KB
    pe_ops = []
    idle = fills = 0.0
    while any(p < NOPS for p in ptr):
        best = None
        best_need = None
        for t in range(NLEAD):
            p = ptr[t]
            if p >= NOPS:
                continue
            if p < NP:
                need = max(arr[('x8', t)], arr[('w8', p)])
            else:
                ki = p - NP
                need = max(arr[('xb', t, ki // CK)], arr[('wb', ki)])
            if need <= t_pe:
                best = t
                break
            if best_need is None or need < best_need:
                best, best_need = t, need
        p = ptr[best]
        if p < NP:
            need = max(arr[('x8', best)], arr[('w8', p)])
        else:
            ki = p - NP
            need = max(arr[('xb', best, ki // CK)], arr[('wb', ki)])
        if need > t_pe:
            gap = need - t_pe
            nf = int(gap * 0.75 / T_FILL)
            if nf > 0:
                pe_ops.append(('fill', nf))
                fills += nf * T_FILL
                t_pe += nf * T_FILL
            idle += max(0.0, need - t_pe)
            t_pe = max(t_pe, need)
        pe_ops.append(('dr', best, p) if p < NP else ('bf', best, p - NP))
        t_pe += T_DR if p < NP else T_BF
        ptr[best] += 1
    return qlists, pe_ops, idle, fills, t_pe


def _build():
    nc = bacc.Bacc(None, target_bir_lowering=False)

    xb = nc.dram_tensor("xb", [MT, P, KB, P], F16, kind="ExternalInput")
    x8 = nc.dram_tensor("x8", [MT, P, NP, 2, P], E4, kind="ExternalInput")
    wb = nc.dram_tensor("wb", [KB * P, O_LOC], F16, kind="ExternalInput")
    w8 = nc.dram_tensor("w8", [NP, P, 2, O_LOC], E4, kind="ExternalInput")
    br = nc.dram_tensor("br", [P, O_LOC], F32, kind="ExternalInput")
    out = nc.dram_tensor("out", [M_TOT, O_LOC], F16, kind="ExternalOutput")

    qlists, pe_ops, idle, fills, lead_end = _lead_schedule()
    print(f"[v17 build] lead sim: idle={idle:.2f}us fills={fills:.2f}us "
          f"lead_end={lead_end:.2f}us")

    with tile.TileContext(nc) as tc:
        with (
            tc.tile_pool(name="const", bufs=1) as const_pool,
            tc.tile_pool(name="xin", bufs=12) as xin_pool,
            tc.tile_pool(name="outs", bufs=4) as out_pool,
            tc.tile_pool(name="psum_mm", bufs=8, space="PSUM") as psum_pool,
        ):
            wb_sb = const_pool.tile([P, KB, O_LOC], F16, name="wb_sb")
            w8_sb = const_pool.tile([P, NP, 2, O_LOC], E4, name="w8_sb")
            bias_sb = const_pool.tile([P, O_LOC], F32, name="bias_sb")

            qeng = [nc.scalar, nc.sync, nc.gpsimd]

            # lead tiles must be allocated before issuing chunked DMAs
            lead_xb = {t: xin_pool.tile([P, KB, P], F16, name="xb_t", tag="xb_t")
                       for t in range(NLEAD)}
            lead_x8 = {t: xin_pool.tile([P, NP, 2, P], E4, name="x8_t", tag="x8_t")
                       for t in range(NLEAD)}
            for q, qitems in enumerate(qlists):
                eng = qeng[q]
                for it in qitems:
                    kind = it[0]
                    if kind == 'x8':
                        eng.dma_start(lead_x8[it[1]][:], x8[it[1]])
                    elif kind == 'xb':
                        t, c = it[1], it[2]
                        eng.dma_start(
                            lead_xb[t][:, c * CK:(c + 1) * CK, :],
                            xb[t, :, c * CK:(c + 1) * CK, :],
                        )
                    elif kind == 'w8':
                        eng.dma_start(w8_sb[:, it[1], :, :], w8[it[1]])
                    elif kind == 'wb':
                        ki = it[1]
                        eng.dma_start(wb_sb[:, ki, :], wb[ki * P:(ki + 1) * P, :])
                    else:
                        eng.dma_start(bias_sb[:], br[:])

            lead_psums = [
                psum_pool.tile([P, O_LOC], F32, name=f"psum_{t}", tag="ps")
                for t in range(NLEAD)
            ]

            # HAM warm-up + gap fillers: matmuls on a zeroed scratch tile.
            # Pre-start they are discarded (start=True clears the bank);
            # mid-chain they accumulate 0.0 — numerically neutral either way.
            scratch = const_pool.tile([P, 2 * P], F16, name="scratch")
            nc.vector.memset(scratch[:], 0)

            started = [False] * NLEAD
            stopped = [False] * NLEAD

            def filler(n):
                # target a bank that is mid-accumulation (or not started):
                # never one already stopped (vector may be reading it).
                tgt = None
                for t in range(NLEAD):
                    if started[t] and not stopped[t]:
                        tgt = t
                        break
                if tgt is None:
                    tgt = next(t for t in range(NLEAD) if not stopped[t])
                st = not started[tgt]  # pre-start bank: plain overwrite is fine
                for _ in range(n):
                    nc.tensor.matmul(
                        lead_psums[tgt][:, :P], scratch[:, :P], scratch[:, P:],
                        start=st, stop=st,
                    )

            filler(NWARM)

            def mm_dr(x8_t, j, psum, start):
                nc.tensor.matmul(
                    psum[:], x8_t[:, j, :, :], w8_sb[:, j, :, :],
                    start=start, stop=False, perf_mode=DR,
                )

            def mm_bf(xb_t, ki, psum):
                nc.tensor.matmul(
                    psum[:], xb_t[:, ki, :], wb_sb[:, ki, :],
                    start=False, stop=(ki == KB - 1),
                )

            def store_out(mt, psum, split=False):
                if not split:
                    o_tile = out_pool.tile([P, O_LOC], F16, name="o_tile", tag="o_tile")
                    nc.vector.tensor_add(out=o_tile[:], in0=psum[:], in1=bias_sb[:])
                    nc.scalar.dma_start(out[mt * P:(mt + 1) * P, :], o_tile[:])
                else:
                    H = O_LOC // 2
                    o_tile = out_pool.tile([P, O_LOC], F16, name="o_tile", tag="o_tile")
                    for h, eng in ((0, nc.gpsimd), (1, nc.sync)):
                        nc.vector.tensor_add(
                            out=o_tile[:, h * H:(h + 1) * H],
                            in0=psum[:, h * H:(h + 1) * H],
                            in1=bias_sb[:, h * H:(h + 1) * H],
                        )
                        eng.dma_start(
                            out[mt * P:(mt + 1) * P, h * H:(h + 1) * H],
                            o_tile[:, h * H:(h + 1) * H],
                        )

            # ---- lead chains: greedy-interleaved PE stream
            done = [0] * NLEAD
            for op in pe_ops:
                if op[0] == 'fill':
                    filler(op[1])
                    continue
                kind, t = op[0], op[1]
                if kind == 'dr':
                    mm_dr(lead_x8[t], op[2], lead_psums[t], start=(op[2] == 0))
                    started[t] = True
                else:
                    mm_bf(lead_xb[t], op[2], lead_psums[t])
                done[t] += 1
                if done[t] == NP + KB:
                    stopped[t] = True
                    store_out(t, lead_psums[t])

            # ---- steady: tile-major; xb on gpsimd (fast queue), x8 on
            # sync, out on scalar.
            for mt in range(NLEAD, MT):
                xb_t = xin_pool.tile([P, KB, P], F16, name="xb_t", tag="xb_t")
                nc.gpsimd.dma_start(xb_t[:], xb[mt])
                x8_t = xin_pool.tile([P, NP, 2, P], E4, name="x8_t", tag="x8_t")
                nc.sync.dma_start(x8_t[:], x8[mt])
                psum = psum_pool.tile([P, O_LOC], F32, name=f"psum_{mt}", tag="ps")
                for j in range(NP):
                    mm_dr(x8_t, j, psum, start=(j == 0))
                for ki in range(KB):
                    mm_bf(xb_t, ki, psum)
                store_out(mt, psum, split=(mt >= MT - 2))
    nc.finalize()
    return nc


def kernel(x, W, bias, lora_A, lora_B):
    x = np.asarray(x, dtype=np.float32)
    W = np.asarray(W, dtype=np.float32)
    bias = np.asarray(bias, dtype=np.float32)
    lora_A = np.asarray(lora_A, dtype=np.float32)
    lora_B = np.asarray(lora_B, dtype=np.float32)

    if "nc" not in _cache:
        _cache["nc"] = _build()
    nc = _cache["nc"]

    Wtot = W + lora_A @ lora_B                      # [out, in] f32
    xr = x.reshape(M_TOT, IN_F)
    KF = KB * P
    # token-side tensors are shared by all 8 cores (pure column sharding)
    xbh = np.ascontiguousarray(
        xr[:, :KF].astype(np.float16).reshape(MT, P, KB, P).transpose(0, 3, 2, 1)
    )
    x8h = np.ascontiguousarray(
        (xr[:, KF:] * (1.0 / FS))
        .astype(ml_dtypes.float8_e4m3fn)
        .reshape(MT, P, NP, 2, P)
        .transpose(0, 4, 2, 3, 1)
    )
    in_maps = []
    for c in range(OG):
        WT = Wtot[c * O_LOC:(c + 1) * O_LOC].T       # [IN_F, O_LOC]
        wbh = np.ascontiguousarray(WT[:KF].astype(np.float16))
        w8h = np.ascontiguousarray(
            (WT[KF:] * FS)
            .astype(ml_dtypes.float8_e4m3fn)
            .reshape(NP, 2, P, O_LOC)
            .transpose(0, 2, 1, 3)
        )
        in_maps.append(
            {
                "xb": xbh,
                "x8": x8h,
                "wb": wbh,
                "w8": w8h,
                "br": np.ascontiguousarray(
                    np.broadcast_to(bias[c * O_LOC:(c + 1) * O_LOC], (P, O_LOC))
                ).astype(np.float32),
            }
        )

    res = run_bass_kernel_spmd(nc, in_maps, core_ids=list(range(8)))

    out = np.empty((M_TOT, OUT_F), dtype=np.float32)
    for c in range(OG):
        out[:, c * O_LOC:(c + 1) * O_LOC] = res.results[c]["out"]
    return out.reshape(BATCH, SEQ, OUT_F)


# revision 7
# speedup vs baseline: 1.0093x; 1.0093x over previous
"""LoRA Linear kernel for Trainium2, 8 NeuronCores — v17.

out = x @ (W + lora_A @ lora_B)^T + bias for x [4,2048,4096],
W [4096,4096], lora_A [4096,16], lora_B [16,4096].

Sharding: pure column-parallel (8-way out_features); every core sees all
8192 tokens and 512 out features.  O_LOC=512 = one PSUM bank per token
tile -> 8 independent accumulation chains, ~2 us tail.

Numerics (numpy-emulated, matches HW to 4 digits): Wtot folded on host;
NF8=8 of 32 k-slices as e4m3 DoubleRow pairs (FS=8 symmetric pow-2
scales), 24 slices in fp16, fp16 output.  rel_l2 = 1.892e-2 < 2e-2.

v17 scheduling (from v16 trace analysis):
- Per-queue DMA rates are asymmetric (measured: scalar ~72 GB/s,
  sync ~88, gpsimd ~165; distinct start offsets).  Lead items are
  assigned earliest-projected-finish across 4 queues (scalar, sync,
  gpsimd, vector) with criticality-ordered priorities; the PE lead
  stream is generated by a build-time greedy that paces 8 chains
  against modeled arrivals.
- Modeled arrival gaps are bridged with zero-matmuls (0 x 0 accumulated
  into a mid-chain PSUM bank adds 0.0): keeps HAM at K=8/8 through the
  lead phase instead of re-throttling to 1.2 GHz.
- Steady phase: xb rides the fast gpsimd queue alone, x8 on sync,
  out on scalar; last two stores are split in half and pushed on two
  queues to shorten the tail.
"""

import ml_dtypes
import numpy as np

import concourse.bacc as bacc
import concourse.mybir as mybir
import concourse.tile as tile
from concourse.bass_utils import run_bass_kernel_spmd

IN_F = 4096
OUT_F = 4096
BATCH, SEQ = 4, 2048
M_TOT = BATCH * SEQ          # 8192 tokens
OG = 8                       # pure out-feature sharding
O_LOC = OUT_F // OG          # 512 out features per core = one PSUM bank
P = 128
KI = IN_F // P               # 32 contraction slices
NF8 = 8                      # k-slices done as e4m3 DoubleRow (even)
KB = KI - NF8                # 24 k-slices at fp16
NP = NF8 // 2                # DoubleRow pairs
MT = M_TOT // P              # 64 token tiles per core
NLEAD = 8                    # lead chains = all 8 PSUM banks
NCHUNK = 6                   # lead xb split into 6 chunks of 4 slices
CK = KB // NCHUNK
FS = 8.0                     # fp8 symmetric scale: x/FS, W*FS
NWARM = 30                   # HAM warm-up matmuls (N=128, dep-free)

F32 = mybir.dt.float32
F16 = mybir.dt.float16
E4 = mybir.dt.float8e4
DR = mybir.MatmulPerfMode.DoubleRow

_cache = {}


def _lead_schedule():
    """Assign lead DMA items to 4 queues (earliest projected finish,
    measured per-queue rates) and greedily pace the 8 lead chains
    against modeled arrivals.  Emits filler markers for modeled gaps.

    Returns (queue_lists, pe_ops):
      queue_lists: per-queue ordered item lists
      pe_ops: ('dr',t,j) | ('bf',t,ki) | ('fill',n)
    """
    # priority-ordered items
    items = []
    items += [('x8', 0), ('w8', 0), ('x8', 1), ('w8', 1), ('w8', 2), ('w8', 3)]
    items += [('x8', t) for t in range(2, NLEAD)]
    wb_next = 0

    def wb_run(n):
        nonlocal wb_next
        out = [('wb', k) for k in range(wb_next, min(wb_next + n, KB))]
        wb_next += len(out)
        return out

    xbc = lambda t: [('xb', t, c) for c in range(NCHUNK)]
    items += xbc(0) + wb_run(2)
    items += xbc(1) + wb_run(2)
    items += xbc(2) + wb_run(3)
    items += xbc(3) + wb_run(3)
    items += xbc(4) + wb_run(3)
    items += xbc(5) + wb_run(4)
    items += xbc(6) + wb_run(4)
    items += xbc(7) + wb_run(KB) + [('bias',)]

    KIB = {'x8': 128, 'w8': 128, 'xb': 256 * 3 // NCHUNK, 'wb': 128, 'bias': 256}
    # conservative per-queue (start us, us-per-KiB): rates are
    # run-variable (HW queues measured 52-90 GB/s, gpsimd 110-165);
    # model the pessimistic end so fillers bridge the spool-up.
    QCFG = [(9.5, 0.01538), (9.5, 0.01538), (11.5, 0.00769)]
    NQ = len(QCFG)
    qt = [s for s, _ in QCFG]
    qlists = [[] for _ in QCFG]
    arr = {}
    for it in items:
        kib = KIB[it[0]]
        fins = [max(qt[q], QCFG[q][0]) + kib * QCFG[q][1] for q in range(NQ)]
        q = min(range(NQ), key=lambda i: fins[i])
        qt[q] = fins[q]
        qlists[q].append(it)
        arr[it] = fins[q]

    # --- PE greedy (first-runnable chain in order; fillers on gaps) ---
    T_DR, T_BF, T_FILL = 0.241, 0.213, 0.107
    t_pe = 7.4 + NWARM * T_FILL
    ptr = [0] * NLEAD
    NOPS = NP + KB
    pe_ops = []
    idle = fills = 0.0
    while any(p < NOPS for p in ptr):
        best = None
        best_need = None
        for t in range(NLEAD):
            p = ptr[t]
            if p >= NOPS:
                continue
            if p < NP:
                need = max(arr[('x8', t)], arr[('w8', p)])
            else:
                ki = p - NP
                need = max(arr[('xb', t, ki // CK)], arr[('wb', ki)])
            if need <= t_pe:
                best = t
                break
            if best_need is None or need < best_need:
                best, best_need = t, need
        p = ptr[best]
        if p < NP:
            need = max(arr[('x8', best)], arr[('w8', p)])
        else:
            ki = p - NP
            need = max(arr[('xb', best, ki // CK)], arr[('wb', ki)])
        if need > t_pe:
            gap = need - t_pe
            nf = int(gap * 0.75 / T_FILL)
            if nf > 0:
                pe_ops.append(('fill', nf))
                fills += nf * T_FILL
                t_pe += nf * T_FILL
            idle += max(0.0, need - t_pe)
            t_pe = max(t_pe, need)
        pe_ops.append(('dr', best, p) if p < NP else ('bf', best, p - NP))
        t_pe += T_DR if p < NP else T_BF
        ptr[best] += 1
    return qlists, pe_ops, idle, fills, t_pe


def _build():
    nc = bacc.Bacc(None, target_bir_lowering=False)

    xb = nc.dram_tensor("xb", [MT, P, KB, P], F16, kind="ExternalInput")
    x8 = nc.dram_tensor("x8", [MT, P, NP, 2, P], E4, kind="ExternalInput")
    wb = nc.dram_tensor("wb", [KB * P, O_LOC], F16, kind="ExternalInput")
    w8 = nc.dram_tensor("w8", [NP, P, 2, O_LOC], E4, kind="ExternalInput")
    br = nc.dram_tensor("br", [P, O_LOC], F32, kind="ExternalInput")
    out = nc.dram_tensor("out", [M_TOT, O_LOC], F16, kind="ExternalOutput")

    qlists, pe_ops, idle, fills, lead_end = _lead_schedule()
    print(f"[v17 build] lead sim: idle={idle:.2f}us fills={fills:.2f}us "
          f"lead_end={lead_end:.2f}us")

    with tile.TileContext(nc) as tc:
        with (
            tc.tile_pool(name="const", bufs=1) as const_pool,
            tc.tile_pool(name="xin", bufs=12) as xin_pool,
            tc.tile_pool(name="outs", bufs=4) as out_pool,
            tc.tile_pool(name="psum_mm", bufs=8, space="PSUM") as psum_pool,
        ):
            wb_sb = const_pool.tile([P, KB, O_LOC], F16, name="wb_sb")
            w8_sb = const_pool.tile([P, NP, 2, O_LOC], E4, name="w8_sb")
            bias_sb = const_pool.tile([P, O_LOC], F32, name="bias_sb")

            qeng = [nc.scalar, nc.sync, nc.gpsimd]

            # lead tiles must be allocated before issuing chunked DMAs
            lead_xb = {t: xin_pool.tile([P, KB, P], F16, name="xb_t", tag="xb_t")
                       for t in range(NLEAD)}
            lead_x8 = {t: xin_pool.tile([P, NP, 2, P], E4, name="x8_t", tag="x8_t")
                       for t in range(NLEAD)}
            for q, qitems in enumerate(qlists):
                eng = qeng[q]
                for it in qitems:
                    kind = it[0]
                    if kind == 'x8':
                        eng.dma_start(lead_x8[it[1]][:], x8[it[1]])
                    elif kind == 'xb':
                        t, c = it[1], it[2]
                        eng.dma_start(
                            lead_xb[t][:, c * CK:(c + 1) * CK, :],
                            xb[t, :, c * CK:(c + 1) * CK, :],
                        )
                    elif kind == 'w8':
                        eng.dma_start(w8_sb[:, it[1], :, :], w8[it[1]])
                    elif kind == 'wb':
                        ki = it[1]
                        eng.dma_start(wb_sb[:, ki, :], wb[ki * P:(ki + 1) * P, :])
                    else:
                        eng.dma_start(bias_sb[:], br[:])

            lead_psums = [
                psum_pool.tile([P, O_LOC], F32, name=f"psum_{t}", tag="ps")
                for t in range(NLEAD)
            ]

            # HAM warm-up + gap fillers: matmuls on a zeroed scratch tile.
            # Pre-start they are discarded (start=True clears the bank);
            # mid-chain they accumulate 0.0 — numerically neutral either way.
            scratch = const_pool.tile([P, 2 * P], F16, name="scratch")
            nc.vector.memset(scratch[:], 0)

            started = [False] * NLEAD
            stopped = [False] * NLEAD

            def filler(n):
                # target a bank that is mid-accumulation (or not started):
                # never one already stopped (vector may be reading it).
                tgt = None
                for t in range(NLEAD):
                    if started[t] and not stopped[t]:
                        tgt = t
                        break
                if tgt is None:
                    tgt = next(t for t in range(NLEAD) if not stopped[t])
                st = not started[tgt]  # pre-start bank: plain overwrite is fine
                for _ in range(n):
                    nc.tensor.matmul(
                        lead_psums[tgt][:, :P], scratch[:, :P], scratch[:, P:],
                        start=st, stop=st,
                    )

            filler(NWARM)

            def mm_dr(x8_t, j, psum, start):
                nc.tensor.matmul(
                    psum[:], x8_t[:, j, :, :], w8_sb[:, j, :, :],
                    start=start, stop=False, perf_mode=DR,
                )

            def mm_bf(xb_t, ki, psum):
                nc.tensor.matmul(
                    psum[:], xb_t[:, ki, :], wb_sb[:, ki, :],
                    start=False, stop=(ki == KB - 1),
                )

            def store_out(mt, psum, split=False):
                if not split:
                    o_tile = out_pool.tile([P, O_LOC], F16, name="o_tile", tag="o_tile")
                    nc.vector.tensor_add(out=o_tile[:], in0=psum[:], in1=bias_sb[:])
                    nc.scalar.dma_start(out[mt * P:(mt + 1) * P, :], o_tile[:])
                else:
                    H = O_LOC // 2
                    o_tile = out_pool.tile([P, O_LOC], F16, name="o_tile", tag="o_tile")
                    for h, eng in ((0, nc.gpsimd), (1, nc.sync)):
                        nc.vector.tensor_add(
                            out=o_tile[:, h * H:(h + 1) * H],
                            in0=psum[:, h * H:(h + 1) * H],
                            in1=bias_sb[:, h * H:(h + 1) * H],
                        )
                        eng.dma_start(
                            out[mt * P:(mt + 1) * P, h * H:(h + 1) * H],
                            o_tile[:, h * H:(h + 1) * H],
                        )

            # ---- lead chains: greedy-interleaved PE stream
            done = [0] * NLEAD
            for op in pe_ops:
                if op[0] == 'fill':
                    filler(op[1])
                    continue
                kind, t = op[0], op[1]
                if kind == 'dr':
                    mm_dr(lead_x8[t], op[2], lead_psums[t], start=(op[2] == 0))
                    started[t] = True
                else:
                    mm_bf(lead_xb[t], op[2], lead_psums[t])
                done[t] += 1
                if done[t] == NP + KB:
                    stopped[t] = True
                    store_out(t, lead_psums[t])

            # ---- steady: tile-major; each xb is split half/half across
            # sync+gpsimd (balances run-variable queue rates), x8 on
            # scalar (with the out stream; both small).
            KH = KB // 2
            for mt in range(NLEAD, MT):
                xb_t = xin_pool.tile([P, KB, P], F16, name="xb_t", tag="xb_t")
                nc.sync.dma_start(xb_t[:, :KH, :], xb[mt, :, :KH, :])
                nc.gpsimd.dma_start(xb_t[:, KH:, :], xb[mt, :, KH:, :])
                x8_t = xin_pool.tile([P, NP, 2, P], E4, name="x8_t", tag="x8_t")
                nc.scalar.dma_start(x8_t[:], x8[mt])
                psum = psum_pool.tile([P, O_LOC], F32, name=f"psum_{mt}", tag="ps")
                for j in range(NP):
                    mm_dr(x8_t, j, psum, start=(j == 0))
                for ki in range(KB):
                    mm_bf(xb_t, ki, psum)
                store_out(mt, psum, split=(mt >= MT - 2))
    nc.finalize()
    return nc


def kernel(x, W, bias, lora_A, lora_B):
    x = np.asarray(x, dtype=np.float32)
    W = np.asarray(W, dtype=np.float32)
    bias = np.asarray(bias, dtype=np.float32)
    lora_A = np.asarray(lora_A, dtype=np.float32)
    lora_B = np.asarray(lora_B, dtype=np.float32)

    if "nc" not in _cache:
        _cache["nc"] = _build()
    nc = _cache["nc"]

    Wtot = W + lora_A @ lora_B                      # [out, in] f32
    xr = x.reshape(M_TOT, IN_F)
    KF = KB * P
    # token-side tensors are shared by all 8 cores (pure column sharding)
    xbh = np.ascontiguousarray(
        xr[:, :KF].astype(np.float16).reshape(MT, P, KB, P).transpose(0, 3, 2, 1)
    )
    x8h = np.ascontiguousarray(
        (xr[:, KF:] * (1.0 / FS))
        .astype(ml_dtypes.float8_e4m3fn)
        .reshape(MT, P, NP, 2, P)
        .transpose(0, 4, 2, 3, 1)
    )
    in_maps = []
    for c in range(OG):
        WT = Wtot[c * O_LOC:(c + 1) * O_LOC].T       # [IN_F, O_LOC]
        wbh = np.ascontiguousarray(WT[:KF].astype(np.float16))
        w8h = np.ascontiguousarray(
            (WT[KF:] * FS)
            .astype(ml_dtypes.float8_e4m3fn)
            .reshape(NP, 2, P, O_LOC)
            .transpose(0, 2, 1, 3)
        )
        in_maps.append(
            {
                "xb": xbh,
                "x8": x8h,
                "wb": wbh,
                "w8": w8h,
                "br": np.ascontiguousarray(
                    np.broadcast_to(bias[c * O_LOC:(c + 1) * O_LOC], (P, O_LOC))
                ).astype(np.float32),
            }
        )

    res = run_bass_kernel_spmd(nc, in_maps, core_ids=list(range(8)))

    out = np.empty((M_TOT, OUT_F), dtype=np.float32)
    for c in range(OG):
        out[:, c * O_LOC:(c + 1) * O_LOC] = res.results[c]["out"]
    return out.reshape(BATCH, SEQ, OUT_F)


# revision 8
# speedup vs baseline: 1.0182x; 1.0088x over previous
"""LoRA Linear kernel for Trainium2, 8 NeuronCores — v17.

out = x @ (W + lora_A @ lora_B)^T + bias for x [4,2048,4096],
W [4096,4096], lora_A [4096,16], lora_B [16,4096].

Sharding: pure column-parallel (8-way out_features); every core sees all
8192 tokens and 512 out features.  O_LOC=512 = one PSUM bank per token
tile -> 8 independent accumulation chains, ~2 us tail.

Numerics (numpy-emulated, matches HW to 4 digits): Wtot folded on host;
NF8=8 of 32 k-slices as e4m3 DoubleRow pairs (FS=8 symmetric pow-2
scales), 24 slices in fp16, fp16 output.  rel_l2 = 1.892e-2 < 2e-2.

v17 scheduling (from v16 trace analysis):
- Per-queue DMA rates are asymmetric (measured: scalar ~72 GB/s,
  sync ~88, gpsimd ~165; distinct start offsets).  Lead items are
  assigned earliest-projected-finish across 4 queues (scalar, sync,
  gpsimd, vector) with criticality-ordered priorities; the PE lead
  stream is generated by a build-time greedy that paces 8 chains
  against modeled arrivals.
- Modeled arrival gaps are bridged with zero-matmuls (0 x 0 accumulated
  into a mid-chain PSUM bank adds 0.0): keeps HAM at K=8/8 through the
  lead phase instead of re-throttling to 1.2 GHz.
- Steady phase: xb rides the fast gpsimd queue alone, x8 on sync,
  out on scalar; last two stores are split in half and pushed on two
  queues to shorten the tail.
"""

import ml_dtypes
import numpy as np

import concourse.bacc as bacc
import concourse.mybir as mybir
import concourse.tile as tile
from concourse.bass_utils import run_bass_kernel_spmd

IN_F = 4096
OUT_F = 4096
BATCH, SEQ = 4, 2048
M_TOT = BATCH * SEQ          # 8192 tokens
OG = 8                       # pure out-feature sharding
O_LOC = OUT_F // OG          # 512 out features per core = one PSUM bank
P = 128
KI = IN_F // P               # 32 contraction slices
NF8 = 8                      # k-slices done as e4m3 DoubleRow (even)
KB = KI - NF8                # 24 k-slices at fp16
NP = NF8 // 2                # DoubleRow pairs
MT = M_TOT // P              # 64 token tiles per core
NLEAD = 8                    # lead chains = all 8 PSUM banks
NCHUNK = 6                   # lead xb split into 6 chunks of 4 slices
CK = KB // NCHUNK
FS = 8.0                     # fp8 symmetric scale: x/FS, W*FS
NWARM = 30                   # HAM warm-up matmuls (N=128, dep-free)

F32 = mybir.dt.float32
F16 = mybir.dt.float16
E4 = mybir.dt.float8e4
DR = mybir.MatmulPerfMode.DoubleRow

_cache = {}


def _lead_schedule():
    """Assign lead DMA items to 4 queues (earliest projected finish,
    measured per-queue rates) and greedily pace the 8 lead chains
    against modeled arrivals.  Emits filler markers for modeled gaps.

    Returns (queue_lists, pe_ops):
      queue_lists: per-queue ordered item lists
      pe_ops: ('dr',t,j) | ('bf',t,ki) | ('fill',n)
    """
    # priority-ordered items
    items = []
    items += [('x8', 0), ('w8', 0), ('x8', 1), ('w8', 1), ('w8', 2), ('w8', 3)]
    items += [('x8', t) for t in range(2, NLEAD)]
    wb_next = 0

    def wb_run(n):
        nonlocal wb_next
        out = [('wb', k) for k in range(wb_next, min(wb_next + n, KB))]
        wb_next += len(out)
        return out

    xbc = lambda t: [('xb', t, c) for c in range(NCHUNK)]
    items += xbc(0) + wb_run(2)
    items += xbc(1) + wb_run(2)
    items += xbc(2) + wb_run(3)
    items += xbc(3) + wb_run(3)
    items += xbc(4) + wb_run(3)
    items += xbc(5) + wb_run(4)
    items += xbc(6) + wb_run(4)
    items += xbc(7) + wb_run(KB) + [('bias',)]

    KIB = {'x8': 128, 'w8': 128, 'xb': 256 * 3 // NCHUNK, 'wb': 128, 'bias': 256}
    # conservative per-queue (start us, us-per-KiB): rates are
    # run-variable (HW queues measured 52-90 GB/s, gpsimd 110-165);
    # model the pessimistic end so fillers bridge the spool-up.
    QCFG = [(9.5, 0.01538), (9.5, 0.01538), (11.5, 0.00769)]
    NQ = len(QCFG)
    qt = [s for s, _ in QCFG]
    qlists = [[] for _ in QCFG]
    arr = {}
    for it in items:
        kib = KIB[it[0]]
        fins = [max(qt[q], QCFG[q][0]) + kib * QCFG[q][1] for q in range(NQ)]
        q = min(range(NQ), key=lambda i: fins[i])
        qt[q] = fins[q]
        qlists[q].append(it)
        arr[it] = fins[q]

    # --- PE greedy (first-runnable chain in order; fillers on gaps) ---
    T_DR, T_BF, T_FILL = 0.241, 0.213, 0.107
    t_pe = 7.4 + NWARM * T_FILL
    ptr = [0] * NLEAD
    NOPS = NP + KB
    pe_ops = []
    idle = fills = 0.0
    while any(p < NOPS for p in ptr):
        best = None
        best_need = None
        for t in range(NLEAD):
            p = ptr[t]
            if p >= NOPS:
                continue
            if p < NP:
                need = max(arr[('x8', t)], arr[('w8', p)])
            else:
                ki = p - NP
                need = max(arr[('xb', t, ki // CK)], arr[('wb', ki)])
            if need <= t_pe:
                best = t
                break
            if best_need is None or need < best_need:
                best, best_need = t, need
        p = ptr[best]
        if p < NP:
            need = max(arr[('x8', best)], arr[('w8', p)])
        else:
            ki = p - NP
            need = max(arr[('xb', best, ki // CK)], arr[('wb', ki)])
        if need > t_pe:
            gap = need - t_pe
            nf = int(gap * 0.75 / T_FILL)
            if nf > 0:
                pe_ops.append(('fill', nf))
                fills += nf * T_FILL
                t_pe += nf * T_FILL
            idle += max(0.0, need - t_pe)
            t_pe = max(t_pe, need)
        pe_ops.append(('dr', best, p) if p < NP else ('bf', best, p - NP))
        t_pe += T_DR if p < NP else T_BF
        ptr[best] += 1
    return qlists, pe_ops, idle, fills, t_pe


def _build():
    nc = bacc.Bacc(None, target_bir_lowering=False)

    xb = nc.dram_tensor("xb", [MT, P, KB, P], F16, kind="ExternalInput")
    x8 = nc.dram_tensor("x8", [MT, P, NP, 2, P], E4, kind="ExternalInput")
    wb = nc.dram_tensor("wb", [KB * P, O_LOC], F16, kind="ExternalInput")
    w8 = nc.dram_tensor("w8", [NP, P, 2, O_LOC], E4, kind="ExternalInput")
    br = nc.dram_tensor("br", [P, O_LOC], F32, kind="ExternalInput")
    out = nc.dram_tensor("out", [M_TOT, O_LOC], F16, kind="ExternalOutput")

    qlists, pe_ops, idle, fills, lead_end = _lead_schedule()
    print(f"[v17 build] lead sim: idle={idle:.2f}us fills={fills:.2f}us "
          f"lead_end={lead_end:.2f}us")

    with tile.TileContext(nc) as tc:
        with (
            tc.tile_pool(name="const", bufs=1) as const_pool,
            tc.tile_pool(name="xin", bufs=12) as xin_pool,
            tc.tile_pool(name="outs", bufs=4) as out_pool,
            tc.tile_pool(name="psum_mm", bufs=8, space="PSUM") as psum_pool,
        ):
            wb_sb = const_pool.tile([P, KB, O_LOC], F16, name="wb_sb")
            w8_sb = const_pool.tile([P, NP, 2, O_LOC], E4, name="w8_sb")
            bias_sb = const_pool.tile([P, O_LOC], F32, name="bias_sb")

            qeng = [nc.scalar, nc.sync, nc.gpsimd]

            # lead tiles must be allocated before issuing chunked DMAs
            lead_xb = {t: xin_pool.tile([P, KB, P], F16, name="xb_t", tag="xb_t")
                       for t in range(NLEAD)}
            lead_x8 = {t: xin_pool.tile([P, NP, 2, P], E4, name="x8_t", tag="x8_t")
                       for t in range(NLEAD)}
            for q, qitems in enumerate(qlists):
                eng = qeng[q]
                for it in qitems:
                    kind = it[0]
                    if kind == 'x8':
                        eng.dma_start(lead_x8[it[1]][:], x8[it[1]])
                    elif kind == 'xb':
                        t, c = it[1], it[2]
                        eng.dma_start(
                            lead_xb[t][:, c * CK:(c + 1) * CK, :],
                            xb[t, :, c * CK:(c + 1) * CK, :],
                        )
                    elif kind == 'w8':
                        eng.dma_start(w8_sb[:, it[1], :, :], w8[it[1]])
                    elif kind == 'wb':
                        ki = it[1]
                        eng.dma_start(wb_sb[:, ki, :], wb[ki * P:(ki + 1) * P, :])
                    else:
                        eng.dma_start(bias_sb[:], br[:])

            lead_psums = [
                psum_pool.tile([P, O_LOC], F32, name=f"psum_{t}", tag="ps")
                for t in range(NLEAD)
            ]

            # HAM warm-up + gap fillers: matmuls on a zeroed scratch tile.
            # Pre-start they are discarded (start=True clears the bank);
            # mid-chain they accumulate 0.0 — numerically neutral either way.
            scratch = const_pool.tile([P, 2 * P], F16, name="scratch")
            nc.vector.memset(scratch[:], 0)

            started = [False] * NLEAD
            stopped = [False] * NLEAD

            def filler(n):
                # target a bank that is mid-accumulation (or not started):
                # never one already stopped (vector may be reading it).
                tgt = None
                for t in range(NLEAD):
                    if started[t] and not stopped[t]:
                        tgt = t
                        break
                if tgt is None:
                    tgt = next(t for t in range(NLEAD) if not stopped[t])
                st = not started[tgt]  # pre-start bank: plain overwrite is fine
                for _ in range(n):
                    nc.tensor.matmul(
                        lead_psums[tgt][:, :P], scratch[:, :P], scratch[:, P:],
                        start=st, stop=st,
                    )

            filler(NWARM)

            def mm_dr(x8_t, j, psum, start):
                nc.tensor.matmul(
                    psum[:], x8_t[:, j, :, :], w8_sb[:, j, :, :],
                    start=start, stop=False, perf_mode=DR,
                )

            def mm_bf(xb_t, ki, psum):
                nc.tensor.matmul(
                    psum[:], xb_t[:, ki, :], wb_sb[:, ki, :],
                    start=False, stop=(ki == KB - 1),
                )

            def store_out(mt, psum, split=False):
                # (split stores measured slower: extra out-queues at program
                # end inflate the engine-drain teardown by ~7 us)
                o_tile = out_pool.tile([P, O_LOC], F16, name="o_tile", tag="o_tile")
                nc.vector.tensor_add(out=o_tile[:], in0=psum[:], in1=bias_sb[:])
                nc.scalar.dma_start(out[mt * P:(mt + 1) * P, :], o_tile[:])

            # ---- lead chains: greedy-interleaved PE stream
            done = [0] * NLEAD
            for op in pe_ops:
                if op[0] == 'fill':
                    filler(op[1])
                    continue
                kind, t = op[0], op[1]
                if kind == 'dr':
                    mm_dr(lead_x8[t], op[2], lead_psums[t], start=(op[2] == 0))
                    started[t] = True
                else:
                    mm_bf(lead_xb[t], op[2], lead_psums[t])
                done[t] += 1
                if done[t] == NP + KB:
                    stopped[t] = True
                    store_out(t, lead_psums[t])

            # ---- steady: tile-major; each xb is split half/half across
            # sync+gpsimd (balances run-variable queue rates), x8 on
            # scalar (with the out stream; both small).
            KH = KB // 2
            for mt in range(NLEAD, MT):
                xb_t = xin_pool.tile([P, KB, P], F16, name="xb_t", tag="xb_t")
                nc.sync.dma_start(xb_t[:, :KH, :], xb[mt, :, :KH, :])
                nc.gpsimd.dma_start(xb_t[:, KH:, :], xb[mt, :, KH:, :])
                x8_t = xin_pool.tile([P, NP, 2, P], E4, name="x8_t", tag="x8_t")
                nc.scalar.dma_start(x8_t[:], x8[mt])
                psum = psum_pool.tile([P, O_LOC], F32, name=f"psum_{mt}", tag="ps")
                for j in range(NP):
                    mm_dr(x8_t, j, psum, start=(j == 0))
                for ki in range(KB):
                    mm_bf(xb_t, ki, psum)
                store_out(mt, psum, split=(mt >= MT - 2))
    nc.finalize()
    return nc


def kernel(x, W, bias, lora_A, lora_B):
    x = np.asarray(x, dtype=np.float32)
    W = np.asarray(W, dtype=np.float32)
    bias = np.asarray(bias, dtype=np.float32)
    lora_A = np.asarray(lora_A, dtype=np.float32)
    lora_B = np.asarray(lora_B, dtype=np.float32)

    if "nc" not in _cache:
        _cache["nc"] = _build()
    nc = _cache["nc"]

    Wtot = W + lora_A @ lora_B                      # [out, in] f32
    xr = x.reshape(M_TOT, IN_F)
    KF = KB * P
    # token-side tensors are shared by all 8 cores (pure column sharding)
    xbh = np.ascontiguousarray(
        xr[:, :KF].astype(np.float16).reshape(MT, P, KB, P).transpose(0, 3, 2, 1)
    )
    x8h = np.ascontiguousarray(
        (xr[:, KF:] * (1.0 / FS))
        .astype(ml_dtypes.float8_e4m3fn)
        .reshape(MT, P, NP, 2, P)
        .transpose(0, 4, 2, 3, 1)
    )
    in_maps = []
    for c in range(OG):
        WT = Wtot[c * O_LOC:(c + 1) * O_LOC].T       # [IN_F, O_LOC]
        wbh = np.ascontiguousarray(WT[:KF].astype(np.float16))
        w8h = np.ascontiguousarray(
            (WT[KF:] * FS)
            .astype(ml_dtypes.float8_e4m3fn)
            .reshape(NP, 2, P, O_LOC)
            .transpose(0, 2, 1, 3)
        )
        in_maps.append(
            {
                "xb": xbh,
                "x8": x8h,
                "wb": wbh,
                "w8": w8h,
                "br": np.ascontiguousarray(
                    np.broadcast_to(bias[c * O_LOC:(c + 1) * O_LOC], (P, O_LOC))
                ).astype(np.float32),
            }
        )

    res = run_bass_kernel_spmd(nc, in_maps, core_ids=list(range(8)))

    out = np.empty((M_TOT, OUT_F), dtype=np.float32)
    for c in range(OG):
        out[:, c * O_LOC:(c + 1) * O_LOC] = res.results[c]["out"]
    return out.reshape(BATCH, SEQ, OUT_F)


# revision 13
# speedup vs baseline: 1.0373x; 1.0188x over previous
"""LoRA Linear kernel for Trainium2, 8 NeuronCores — v17.

out = x @ (W + lora_A @ lora_B)^T + bias for x [4,2048,4096],
W [4096,4096], lora_A [4096,16], lora_B [16,4096].

Sharding: pure column-parallel (8-way out_features); every core sees all
8192 tokens and 512 out features.  O_LOC=512 = one PSUM bank per token
tile -> 8 independent accumulation chains, ~2 us tail.

Numerics (numpy-emulated, matches HW to 4 digits): Wtot folded on host;
NF8=8 of 32 k-slices as e4m3 DoubleRow pairs (FS=8 symmetric pow-2
scales), 24 slices in fp16, fp16 output.  rel_l2 = 1.892e-2 < 2e-2.

v17 scheduling (from v16 trace analysis):
- Per-queue DMA rates are asymmetric (measured: scalar ~72 GB/s,
  sync ~88, gpsimd ~165; distinct start offsets).  Lead items are
  assigned earliest-projected-finish across 4 queues (scalar, sync,
  gpsimd, vector) with criticality-ordered priorities; the PE lead
  stream is generated by a build-time greedy that paces 8 chains
  against modeled arrivals.
- Modeled arrival gaps are bridged with zero-matmuls (0 x 0 accumulated
  into a mid-chain PSUM bank adds 0.0): keeps HAM at K=8/8 through the
  lead phase instead of re-throttling to 1.2 GHz.
- Steady phase: xb rides the fast gpsimd queue alone, x8 on sync,
  out on scalar; last two stores are split in half and pushed on two
  queues to shorten the tail.
"""

import ml_dtypes
import numpy as np

import concourse.bacc as bacc
import concourse.mybir as mybir
import concourse.tile as tile
from concourse.bass_utils import run_bass_kernel_spmd

IN_F = 4096
OUT_F = 4096
BATCH, SEQ = 4, 2048
M_TOT = BATCH * SEQ          # 8192 tokens
OG = 8                       # pure out-feature sharding
O_LOC = OUT_F // OG          # 512 out features per core = one PSUM bank
P = 128
KI = IN_F // P               # 32 contraction slices
NF8 = 8                      # k-slices done as e4m3 DoubleRow (even)
KB = KI - NF8                # 24 k-slices at fp16
NP = NF8 // 2                # DoubleRow pairs
MT = M_TOT // P              # 64 token tiles per core
NLEAD = 8                    # lead chains = all 8 PSUM banks
NCHUNK = 6                   # lead xb split into 6 chunks of 4 slices
CK = KB // NCHUNK
FS = 8.0                     # fp8 symmetric scale: x/FS, W*FS
NWARM = 30                   # HAM warm-up matmuls (N=128, dep-free)

F32 = mybir.dt.float32
F16 = mybir.dt.float16
E4 = mybir.dt.float8e4
DR = mybir.MatmulPerfMode.DoubleRow

_cache = {}


def _lead_schedule():
    """Assign lead DMA items to 4 queues (earliest projected finish,
    measured per-queue rates) and greedily pace the 8 lead chains
    against modeled arrivals.  Emits filler markers for modeled gaps.

    Returns (queue_lists, pe_ops):
      queue_lists: per-queue ordered item lists
      pe_ops: ('dr',t,j) | ('bf',t,ki) | ('fill',n)
    """
    # priority-ordered items
    items = []
    items += [('x8', 0), ('w8', 0), ('x8', 1), ('w8', 1), ('w8', 2), ('w8', 3)]
    items += [('x8', t) for t in range(2, NLEAD)]
    wb_next = 0

    def wb_run(n):
        nonlocal wb_next
        out = [('wb', k) for k in range(wb_next, min(wb_next + n, KB))]
        wb_next += len(out)
        return out

    xbc = lambda t: [('xb', t, c) for c in range(NCHUNK)]
    items += xbc(0) + wb_run(2)
    items += xbc(1) + wb_run(2)
    items += xbc(2) + wb_run(3)
    items += xbc(3) + wb_run(3)
    items += xbc(4) + wb_run(3)
    items += xbc(5) + wb_run(4)
    items += xbc(6) + wb_run(4)
    items += xbc(7) + wb_run(KB) + [('bias',)]

    KIB = {'x8': 128, 'w8': 128, 'xb': 256 * 3 // NCHUNK, 'wb': 128, 'bias': 256}
    # pessimistic per-queue (start us, us-per-KiB): rates are
    # run-variable (HW queues measured 52-90 GB/s, gpsimd 110-165) and
    # worst-core span is the metric, so model the slow end: fillers
    # convert would-be stalls into warm PE time on unlucky cores.
    QCFG = [(10.0, 0.0161), (10.0, 0.0161), (12.0, 0.0080)]
    NQ = len(QCFG)
    qt = [s for s, _ in QCFG]
    qlists = [[] for _ in QCFG]
    arr = {}
    for it in items:
        kib = KIB[it[0]]
        fins = [max(qt[q], QCFG[q][0]) + kib * QCFG[q][1] for q in range(NQ)]
        q = min(range(NQ), key=lambda i: fins[i])
        qt[q] = fins[q]
        qlists[q].append(it)
        arr[it] = fins[q]

    # --- PE greedy (first-runnable chain in order; fillers on gaps) ---
    T_DR, T_BF, T_FILL = 0.241, 0.213, 0.107
    t_pe = 7.4 + NWARM * T_FILL
    ptr = [0] * NLEAD
    NOPS = NP + KB
    pe_ops = []
    idle = fills = 0.0
    while any(p < NOPS for p in ptr):
        best = None
        best_need = None
        for t in range(NLEAD):
            p = ptr[t]
            if p >= NOPS:
                continue
            if p < NP:
                need = max(arr[('x8', t)], arr[('w8', p)])
            else:
                ki = p - NP
                need = max(arr[('xb', t, ki // CK)], arr[('wb', ki)])
            if need <= t_pe:
                best = t
                break
            if best_need is None or need < best_need:
                best, best_need = t, need
        p = ptr[best]
        if p < NP:
            need = max(arr[('x8', best)], arr[('w8', p)])
        else:
            ki = p - NP
            need = max(arr[('xb', best, ki // CK)], arr[('wb', ki)])
        if need > t_pe:
            gap = need - t_pe
            nf = int(gap * 0.85 / T_FILL)
            if nf > 0:
                pe_ops.append(('fill', nf))
                fills += nf * T_FILL
                t_pe += nf * T_FILL
            idle += max(0.0, need - t_pe)
            t_pe = max(t_pe, need)
        pe_ops.append(('dr', best, p) if p < NP else ('bf', best, p - NP))
        t_pe += T_DR if p < NP else T_BF
        ptr[best] += 1
    return qlists, pe_ops, idle, fills, t_pe


def _build():
    nc = bacc.Bacc(None, target_bir_lowering=False)

    xb = nc.dram_tensor("xb", [MT, P, KB, P], F16, kind="ExternalInput")
    x8 = nc.dram_tensor("x8", [MT, P, NP, 2, P], E4, kind="ExternalInput")
    wb = nc.dram_tensor("wb", [KB * P, O_LOC], F16, kind="ExternalInput")
    w8 = nc.dram_tensor("w8", [NP, P, 2, O_LOC], E4, kind="ExternalInput")
    br = nc.dram_tensor("br", [P, O_LOC], F32, kind="ExternalInput")
    out = nc.dram_tensor("out", [M_TOT, O_LOC], F16, kind="ExternalOutput")

    qlists, pe_ops, idle, fills, lead_end = _lead_schedule()
    print(f"[v17 build] lead sim: idle={idle:.2f}us fills={fills:.2f}us "
          f"lead_end={lead_end:.2f}us")

    with tile.TileContext(nc) as tc:
        with (
            tc.tile_pool(name="const", bufs=1) as const_pool,
            tc.tile_pool(name="xin", bufs=12) as xin_pool,
            tc.tile_pool(name="outs", bufs=4) as out_pool,
            tc.tile_pool(name="psum_mm", bufs=8, space="PSUM") as psum_pool,
        ):
            wb_sb = const_pool.tile([P, KB, O_LOC], F16, name="wb_sb")
            w8_sb = const_pool.tile([P, NP, 2, O_LOC], E4, name="w8_sb")
            bias_sb = const_pool.tile([P, O_LOC], F32, name="bias_sb")

            qeng = [nc.scalar, nc.sync, nc.gpsimd]

            # lead tiles must be allocated before issuing chunked DMAs
            lead_xb = {t: xin_pool.tile([P, KB, P], F16, name="xb_t", tag="xb_t")
                       for t in range(NLEAD)}
            lead_x8 = {t: xin_pool.tile([P, NP, 2, P], E4, name="x8_t", tag="x8_t")
                       for t in range(NLEAD)}
            for q, qitems in enumerate(qlists):
                eng = qeng[q]
                for it in qitems:
                    kind = it[0]
                    if kind == 'x8':
                        eng.dma_start(lead_x8[it[1]][:], x8[it[1]])
                    elif kind == 'xb':
                        t, c = it[1], it[2]
                        eng.dma_start(
                            lead_xb[t][:, c * CK:(c + 1) * CK, :],
                            xb[t, :, c * CK:(c + 1) * CK, :],
                        )
                    elif kind == 'w8':
                        eng.dma_start(w8_sb[:, it[1], :, :], w8[it[1]])
                    elif kind == 'wb':
                        ki = it[1]
                        eng.dma_start(wb_sb[:, ki, :], wb[ki * P:(ki + 1) * P, :])
                    else:
                        eng.dma_start(bias_sb[:], br[:])

            # prefetch the first steady tiles inside the lead DMA lists so
            # the lead->steady handover has no arrival cliff
            NPRE = 3
            KH = KB // 2
            for mt in range(NLEAD, NLEAD + NPRE):
                xb_t = xin_pool.tile([P, KB, P], F16, name="xb_t", tag="xb_t")
                nc.sync.dma_start(xb_t[:, :KH, :], xb[mt, :, :KH, :])
                nc.gpsimd.dma_start(xb_t[:, KH:, :], xb[mt, :, KH:, :])
                x8_t = xin_pool.tile([P, NP, 2, P], E4, name="x8_t", tag="x8_t")
                nc.scalar.dma_start(x8_t[:], x8[mt])
                lead_xb[mt], lead_x8[mt] = xb_t, x8_t

            lead_psums = [
                psum_pool.tile([P, O_LOC], F32, name=f"psum_{t}", tag="ps")
                for t in range(NLEAD)
            ]

            # HAM warm-up + gap fillers: matmuls on a zeroed scratch tile.
            # Pre-start they are discarded (start=True clears the bank);
            # mid-chain they accumulate 0.0 — numerically neutral either way.
            scratch = const_pool.tile([P, 2 * P], F16, name="scratch")
            nc.vector.memset(scratch[:], 0)

            started = [False] * NLEAD
            stopped = [False] * NLEAD

            def filler(n):
                # target a bank that is mid-accumulation (or not started):
                # never one already stopped (vector may be reading it).
                tgt = None
                for t in range(NLEAD):
                    if started[t] and not stopped[t]:
                        tgt = t
                        break
                if tgt is None:
                    tgt = next(t for t in range(NLEAD) if not stopped[t])
                st = not started[tgt]  # pre-start bank: plain overwrite is fine
                for _ in range(n):
                    nc.tensor.matmul(
                        lead_psums[tgt][:, :P], scratch[:, :P], scratch[:, P:],
                        start=st, stop=st,
                    )

            filler(NWARM)

            def mm_dr(x8_t, j, psum, start):
                nc.tensor.matmul(
                    psum[:], x8_t[:, j, :, :], w8_sb[:, j, :, :],
                    start=start, stop=False, perf_mode=DR,
                )

            def mm_bf(xb_t, ki, psum):
                nc.tensor.matmul(
                    psum[:], xb_t[:, ki, :], wb_sb[:, ki, :],
                    start=False, stop=(ki == KB - 1),
                )

            def store_out(mt, psum, split=False):
                # (split stores measured slower: extra out-queues at program
                # end inflate the engine-drain teardown by ~7 us)
                o_tile = out_pool.tile([P, O_LOC], F16, name="o_tile", tag="o_tile")
                nc.vector.tensor_add(out=o_tile[:], in0=psum[:], in1=bias_sb[:])
                nc.scalar.dma_start(out[mt * P:(mt + 1) * P, :], o_tile[:])

            # ---- lead chains: greedy-interleaved PE stream
            done = [0] * NLEAD
            for op in pe_ops:
                if op[0] == 'fill':
                    filler(op[1])
                    continue
                kind, t = op[0], op[1]
                if kind == 'dr':
                    mm_dr(lead_x8[t], op[2], lead_psums[t], start=(op[2] == 0))
                    started[t] = True
                else:
                    mm_bf(lead_xb[t], op[2], lead_psums[t])
                done[t] += 1
                if done[t] == NP + KB:
                    stopped[t] = True
                    store_out(t, lead_psums[t])

            # ---- steady: tile-major; each xb is split half/half across
            # sync+gpsimd (balances run-variable queue rates), x8 on
            # scalar (with the out stream; both small).
            for mt in range(NLEAD, MT):
                if mt in lead_xb:
                    xb_t, x8_t = lead_xb[mt], lead_x8[mt]
                else:
                    xb_t = xin_pool.tile([P, KB, P], F16, name="xb_t", tag="xb_t")
                    nc.sync.dma_start(xb_t[:, :KH, :], xb[mt, :, :KH, :])
                    nc.gpsimd.dma_start(xb_t[:, KH:, :], xb[mt, :, KH:, :])
                    x8_t = xin_pool.tile([P, NP, 2, P], E4, name="x8_t", tag="x8_t")
                    nc.scalar.dma_start(x8_t[:], x8[mt])
                psum = psum_pool.tile([P, O_LOC], F32, name=f"psum_{mt}", tag="ps")
                for j in range(NP):
                    mm_dr(x8_t, j, psum, start=(j == 0))
                for ki in range(KB):
                    mm_bf(xb_t, ki, psum)
                store_out(mt, psum, split=(mt >= MT - 2))
    nc.finalize()
    return nc


def kernel(x, W, bias, lora_A, lora_B):
    x = np.asarray(x, dtype=np.float32)
    W = np.asarray(W, dtype=np.float32)
    bias = np.asarray(bias, dtype=np.float32)
    lora_A = np.asarray(lora_A, dtype=np.float32)
    lora_B = np.asarray(lora_B, dtype=np.float32)

    if "nc" not in _cache:
        _cache["nc"] = _build()
    nc = _cache["nc"]

    Wtot = W + lora_A @ lora_B                      # [out, in] f32
    xr = x.reshape(M_TOT, IN_F)
    KF = KB * P
    # token-side tensors are shared by all 8 cores (pure column sharding)
    xbh = np.ascontiguousarray(
        xr[:, :KF].astype(np.float16).reshape(MT, P, KB, P).transpose(0, 3, 2, 1)
    )
    x8h = np.ascontiguousarray(
        (xr[:, KF:] * (1.0 / FS))
        .astype(ml_dtypes.float8_e4m3fn)
        .reshape(MT, P, NP, 2, P)
        .transpose(0, 4, 2, 3, 1)
    )
    in_maps = []
    for c in range(OG):
        WT = Wtot[c * O_LOC:(c + 1) * O_LOC].T       # [IN_F, O_LOC]
        wbh = np.ascontiguousarray(WT[:KF].astype(np.float16))
        w8h = np.ascontiguousarray(
            (WT[KF:] * FS)
            .astype(ml_dtypes.float8_e4m3fn)
            .reshape(NP, 2, P, O_LOC)
            .transpose(0, 2, 1, 3)
        )
        in_maps.append(
            {
                "xb": xbh,
                "x8": x8h,
                "wb": wbh,
                "w8": w8h,
                "br": np.ascontiguousarray(
                    np.broadcast_to(bias[c * O_LOC:(c + 1) * O_LOC], (P, O_LOC))
                ).astype(np.float32),
            }
        )

    res = run_bass_kernel_spmd(nc, in_maps, core_ids=list(range(8)))

    out = np.empty((M_TOT, OUT_F), dtype=np.float32)
    for c in range(OG):
        out[:, c * O_LOC:(c + 1) * O_LOC] = res.results[c]["out"]
    return out.reshape(BATCH, SEQ, OUT_F)
